# revision 37
# baseline (speedup 1.0000x reference)
"""Causal self-attention Trainium2 kernel (8 NeuronCores), v7.

Sharding: data-parallel over batch (2) x tensor-parallel over head groups
(12 heads -> 4 groups of 3). Core c handles batch c//4, head group c%4.
Each core computes its partial projection output (bf16); the host sums
the 4 partials per batch (TP reduce folded into the output gather).

All-bf16 compute.  Measured on HW: matmuls with full 128-row stationary
and 128 moving partitions stream at ~0.42ns/col with LDWEIGHTS fully
hidden inside the previous matmul; narrow shapes (K=64 QK, M=65 AV)
average 1.3-1.5x that.  v7 therefore:
  - pads the AV stationary to M=128 (v tiles stored 128 wide: 64 v cols,
    ones col at 64, zeros above -- extra PSUM rows are exact zeros).
  - pads the denominator-broadcast stationary to M=128 as well.
  - QK starts ~3us in: input DMAs issue the first 512 columns of x
    first, rows 0-1 are chunked at 512, and the prelude only needs two
    qkv groups; the ACT exp table is preloaded during the DMA wait.
  - softmax normalize: DVE den copy (row 64, lane-aligned) -> PE
    broadcast (f32r) into a y-pool PSUM tile -> DVE reciprocal -> DVE
    multiply.  DVE ops never move data across partitions (hardware
    cannot; CoreSim would not catch it).
  - AV(3) pre-accumulated over rows 0..13 in 3 of the 4 y-pool buffers;
    after the last exp only 2 small matmuls per head + proj 12-15
    remain, with half the tail casts on the then-idle ScalarE.
"""

import functools

import numpy as np
import ml_dtypes

import concourse.bass as bass
import concourse.mybir as mybir
import concourse.tile as tile
from concourse import bacc
from concourse.bass_utils import run_bass_kernel_spmd
from concourse.masks import make_upper_triangular

P = 128
B, T, C = 2, 2048, 768
NH, HD = 12, 64
HPG = 3              # heads per core
NT = T // P          # 16 key tiles
NQ = T // 512        # 4 query chunks
QKW = 2 * HPG * HD   # 384 qk channels per core
VW = HPG * HD        # 192 v channels per core
F32 = mybir.dt.float32
F32R = mybir.dt.float32r
BF16 = mybir.dt.bfloat16
BF16NP = ml_dtypes.bfloat16

W = [T - P * j for j in range(NT)]
R3OFF = []
_o = 0
for _j in range(NT):
    R3OFF.append(_o)
    _o += HPG * W[_j]
EXPTOT = _o  # 52224

LAST_RESULT = None


def _emit(nc, tc, xT_d, wqk_d, wv_d, wp_d, out_d):
    from contextlib import ExitStack

    ctx = ExitStack()
    with ctx:
        const = ctx.enter_context(tc.tile_pool(name="const", bufs=1))
        tri_b = const.tile([P, P], BF16)
        make_upper_triangular(nc, tri_b[:], val=1.0, diag=True)
        tri3 = const.tile([P, HPG * P], BF16)
        for h in range(HPG):
            nc.vector.tensor_copy(
                out=tri3[:, h * P : (h + 1) * P], in_=tri_b[:]
            )
        ones_f32 = const.tile([P, HD], F32)
        nc.any.memset(ones_f32[:], 1.0)
        ones64 = const.tile([P, HD], F32R)
        nc.vector.tensor_copy(out=ones64[:], in_=ones_f32[:])

        # ---- weights + xT in SBUF -------------------------------------
        w_pool = ctx.enter_context(tc.tile_pool(name="w", bufs=1))
        wqk_sb = []
        for cc in range(6):
            t = w_pool.tile([P, QKW], BF16, tag=f"wqk{cc}")
            nc.sync.dma_start(t[:], wqk_d[cc * P : (cc + 1) * P, :])
            wqk_sb.append(t)
        x_pool = ctx.enter_context(tc.tile_pool(name="x", bufs=1))
        xT_sb = []
        for cc in range(6):
            t = x_pool.tile([P, T], BF16, tag=f"x{cc}", name=f"x{cc}")
            xT_sb.append(t)
        for cc in range(6):
            nc.sync.dma_start(
                xT_sb[cc][:, 0:1024], xT_d[cc * P : (cc + 1) * P, 0:1024]
            )
        wv_sb = []
        for cc in range(6):
            t = w_pool.tile([P, VW], BF16, tag=f"wv{cc}")
            nc.sync.dma_start(t[:], wv_d[cc * P : (cc + 1) * P, :])
            wv_sb.append(t)
        for cc in range(6):
            nc.sync.dma_start(
                xT_sb[cc][:, 1024:T], xT_d[cc * P : (cc + 1) * P, 1024:T]
            )
        wpA = w_pool.tile([P, C], BF16, tag="wpA")
        nc.sync.dma_start(wpA[:], wp_d[0:P, :])
        wpB = w_pool.tile([HD, C], BF16, tag="wpB")
        nc.sync.dma_start(wpB[:], wp_d[P : P + HD, :])

        # ---- persistent SBUF tensors ----------------------------------
        big = ctx.enter_context(tc.tile_pool(name="big", bufs=1))
        qkT = [
            big.tile([P, T], BF16, tag=f"qkT{m}", name=f"qkT{m}")
            for m in range(3)
        ]
        k2b = big.tile([HD, T], BF16, tag="k2b")
        v_sb = big.tile([P, HPG * NT * (HD + 1)], BF16, tag="vsb")
        nc.any.memset(v_sb[:], 1.0)  # ones cols at 64 mod 65 survive
        vv = v_sb[:].rearrange(
            "p (h t d) -> p h t d", h=HPG, t=NT, d=HD + 1
        )
        exp_sb = big.tile([P, EXPTOT], BF16, tag="exp")
        yT_a = big.tile([P, T], BF16, tag="ya")   # h0 rows 0:64, h1 64:128
        yT_b = big.tile([HD, T], BF16, tag="yb")  # h2

        nrm_pool = ctx.enter_context(tc.tile_pool(name="nrm", bufs=2))
        out_pool = ctx.enter_context(tc.tile_pool(name="outp", bufs=3))

        def blk(j):
            return exp_sb[:, R3OFF[j] : R3OFF[j] + HPG * W[j]].rearrange(
                "p (h n) -> p h n", h=HPG
            )

        q_sl = [(qkT[0], 0), (qkT[0], HD), (qkT[2], 0)]
        k_sl = [(qkT[1], 0), (qkT[1], HD), (k2b, 0)]

        def ydst_of(h):
            return yT_a[0:HD, :] if h == 0 else (
                yT_a[HD:P, :] if h == 1 else yT_b[0:HD, :]
            )

        # ---------------- emission helpers ----------------
        ps_att = ctx.enter_context(
            tc.tile_pool(name="ps_att", bufs=1, space="PSUM")
        )

        def emit_qkv_group(ps_b, m, g):
            ps = ps_b.tile([P, 512], F32, tag="ab", bufs=2)
            for cc in range(6):
                nc.tensor.matmul(
                    ps[:],
                    wqk_sb[cc][:, m * P : (m + 1) * P],
                    xT_sb[cc][:, g * 512 : (g + 1) * 512],
                    start=(cc == 0),
                    stop=(cc == 5),
                )
            nc.vector.tensor_copy(
                out=qkT[m][:, g * 512 : (g + 1) * 512], in_=ps[:]
            )
            if m == 2:
                nc.sync.dma_start(
                    k2b[:, g * 512 : (g + 1) * 512],
                    qkT[2][HD:P, g * 512 : (g + 1) * 512],
                )

        def emit_v_group(ps_b, tt):
            ps = ps_b.tile([P, 512], F32, tag="ab", bufs=2)
            for cc in range(6):
                nc.tensor.matmul(
                    ps[:, 0:VW],
                    xT_sb[cc][:, tt * P : (tt + 1) * P],
                    wv_sb[cc][:, :],
                    start=(cc == 0),
                    stop=(cc == 5),
                )
            dst = vv[:, :, tt, 0:HD]
            src = ps[:, 0:VW].rearrange("p (h d) -> p h d", h=HPG)
            nc.vector.tensor_copy(out=dst, in_=src)

        def emit_qk_unit(j, h, off, cw):
            qlo = P * j + off
            qt, qo = q_sl[h]
            kt, ko = k_sl[h]
            st = ps_att.tile([P, 1024], F32, tag="st", bufs=2)
            for s0 in range(0, cw, 512):
                sw = min(512, cw - s0)
                nc.tensor.matmul(
                    st[:, s0 : s0 + sw],
                    kt[ko : ko + HD, P * j : P * j + P],
                    qt[qo : qo + HD, qlo + s0 : qlo + s0 + sw],
                    start=True,
                    stop=True,
                )
            dst = blk(j)[:, h, off : off + cw]
            nc.scalar.activation(
                dst, st[:, 0:cw], mybir.ActivationFunctionType.Exp,
                scale=0.125,
            )

        def emit_mask(j):
            dg = blk(j)[:, :, 0:P]
            t3 = tri3[:].rearrange("p (h n) -> p h n", h=HPG)
            nc.gpsimd.tensor_mul(out=dg, in0=dg, in1=t3)

        def vtile(h, jj):
            a = (h * NT + jj) * (HD + 1)
            return v_sb[:, a : a + (HD + 1)]

        def emit_av_mm(yq, q, h, jj, is_first, is_last):
            woff = 512 * q - P * jj
            lo = max(0, -woff)
            n = 512 - lo
            rhs = blk(jj)[:, h, woff + lo : woff + lo + n]
            nc.tensor.matmul(
                yq[0 : HD + 1, lo : lo + n],
                vtile(h, jj),
                rhs,
                start=is_first,
                stop=is_last,
            )

        av_state = {}

        def emit_den_copy(key, yq, q, h):
            den = nrm_pool.tile([P, 512], F32R, tag="den")
            nc.vector.tensor_copy(
                out=den[HD : HD + 1, :], in_=yq[HD : HD + 1, 0:512]
            )
            av_state[key] = (yq, den, q, h)

        def emit_norm_fin(key):
            yq, den, q, h = av_state.pop(key)
            bc = ps_c.tile([P, 512], F32, tag="y", bufs=4)
            nc.tensor.matmul(
                bc[0:HD, :],
                ones64[HD : HD + 1, :],
                den[HD : HD + 1, :],
                start=True,
                stop=True,
            )
            bcs = nrm_pool.tile([HD, 512], F32, tag="bcs")
            with nc.allow_low_precision(reason="softmax denom"):
                nc.vector.reciprocal_approx_fast(bcs[:], bc[0:HD, :])
            nc.vector.tensor_mul(
                out=ydst_of(h)[:, 512 * q : 512 * (q + 1)],
                in0=yq[0:HD, 0:512],
                in1=bcs[:],
            )

        def emit_av(ps_c, q, h):
            yq = ps_c.tile([P, 512], F32, tag="y", bufs=4)
            for jj in range(4 * q + 4):
                emit_av_mm(yq, q, h, jj, jj == 0, jj == 4 * q + 3)
            emit_den_copy((q, h), yq, q, h)

        def emit_proj(ps_c, tt, cast_engine="dve"):
            pja = ps_c.tile([P, 512], F32, tag="y", bufs=4)
            pjb = ps_c.tile([P, 512], F32, tag="y", bufs=4)
            ysl = slice(tt * P, (tt + 1) * P)
            nc.tensor.matmul(
                pja[:], yT_a[:, ysl], wpA[:, 0:512], start=True, stop=False
            )
            nc.tensor.matmul(
                pjb[:, 0:256], yT_a[:, ysl], wpA[:, 512:C],
                start=True, stop=False,
            )
            nc.tensor.matmul(
                pja[:], yT_b[:, ysl], wpB[:, 0:512], start=False, stop=True
            )
            nc.tensor.matmul(
                pjb[:, 0:256], yT_b[:, ysl], wpB[:, 512:C],
                start=False, stop=True,
            )
            ot = out_pool.tile([P, C], BF16, tag="o")
            if cast_engine == "act":
                nc.scalar.copy(out=ot[:, 0:512], in_=pja[:])
                nc.scalar.copy(out=ot[:, 512:C], in_=pjb[:, 0:256])
            else:
                nc.vector.tensor_copy(out=ot[:, 0:512], in_=pja[:])
                nc.vector.tensor_copy(out=ot[:, 512:C], in_=pjb[:, 0:256])
            nc.sync.dma_start(out_d[tt * P : (tt + 1) * P, :], ot[:])

        # ---------------- schedule ----------------
        ps_b = tc.alloc_tile_pool(name="ps_b", bufs=1, space="PSUM")
        ps_c = None

        emit_qkv_group(ps_b, 0, 0)
        emit_qkv_group(ps_b, 1, 0)
        emit_qkv_group(ps_b, 2, 0)

        fillers = []
        for g in (1, 2, 3):
            fillers.append(
                (1340, f"m0g{g}", lambda g=g: emit_qkv_group(ps_b, 0, g))
            )
            fillers.append(
                (1340, f"m2g{g}", lambda g=g: emit_qkv_group(ps_b, 2, g))
            )
            fillers.append(
                (1340, f"m1g{g}", lambda g=g: emit_qkv_group(ps_b, 1, g))
            )
        for tt in range(NT):
            fillers.append(
                (480, f"v{tt}", lambda tt=tt: emit_v_group(ps_b, tt))
            )
        fi = 0
        done_tags = set()

        def pop_filler():
            nonlocal fi
            cost, tag, fn = fillers[fi]
            fi += 1
            fn()
            done_tags.add(tag)
            return cost

        def need(tag):
            while tag not in done_tags and fi < len(fillers):
                pop_filler()

        def drain():
            while fi < len(fillers):
                pop_filler()

        for j in range(NT):
            csz = 1024
            offs = []
            o = 0
            while o < W[j]:
                offs.append((o, min(csz, W[j] - o)))
                o += csz
            for off, cw in offs:
                g_need = (P * j + off + cw - 1) // 512
                for g in range(1, g_need + 1):
                    need(f"m0g{g}")
                    need(f"m2g{g}")
                kg = j // 4
                if kg >= 1:
                    need(f"m1g{kg}")
                for h in range(HPG):
                    emit_qk_unit(j, h, off, cw)
                    if off == 0 and h == 2:
                        emit_mask(j)
                    slack = cw * 0.417 + 220
                    while slack > 0 and fi < len(fillers):
                        slack -= pop_filler()
            if j == 3:
                drain()
                ps_b.release()
                ps_c = tc.alloc_tile_pool(name="ps_c", bufs=1, space="PSUM")
            if j in (3, 7, 11):
                q = j // 4
                cost = (4 * q + 2) * 215
                for h in range(HPG):
                    fillers.append(
                        (cost, f"av{q}h{h}",
                         lambda q=q, h=h: emit_av(ps_c, q, h))
                    )
                    if h >= 1:
                        fillers.append(
                            (260, f"nf{q}h{h - 1}",
                             lambda q=q, h=h: emit_norm_fin((q, h - 1)))
                        )
                fillers.append(
                    (260, f"nf{q}h2", lambda q=q: emit_norm_fin((q, 2)))
                )
                for t4 in range(4):
                    fillers.append(
                        (660, f"pj{4 * q + t4}",
                         lambda q=q, t4=t4: emit_proj(ps_c, 4 * q + t4))
                    )
            if j == 13:
                drain()
                # pre-accumulate AV(3) over rows 0..13 in 3 y-pool bufs
                yq3 = []
                for h in range(HPG):
                    yq = ps_c.tile([P, 512], F32, tag="y", bufs=4)
                    for jj in range(14):
                        emit_av_mm(yq, 3, h, jj, jj == 0, False)
                    yq3.append(yq)

        # ---- tail: finish AV(3), last projection tiles ----
        for h in range(HPG):
            emit_av_mm(yq3[h], 3, h, 14, False, False)
            emit_av_mm(yq3[h], 3, h, 15, False, True)
            emit_den_copy((3, h), yq3[h], 3, h)
        for h in range(HPG):
            emit_norm_fin((3, h))
        emit_proj(ps_c, 12, cast_engine="dve")
        emit_proj(ps_c, 13, cast_engine="act")
        emit_proj(ps_c, 14, cast_engine="dve")
        emit_proj(ps_c, 15, cast_engine="act")
        ps_c.release()


@functools.cache
def _build():
    nc = bacc.Bacc(
        "TRN2",
        target_bir_lowering=False,
        debug=False,
        enable_asserts=False,
        num_devices=8,
    )
    xT_d = nc.dram_tensor("xt", [C, T], BF16, kind="ExternalInput").ap()
    wqk_d = nc.dram_tensor("wqk", [C, QKW], BF16, kind="ExternalInput").ap()
    wv_d = nc.dram_tensor("wv", [C, VW], BF16, kind="ExternalInput").ap()
    wp_d = nc.dram_tensor("wp", [VW, C], BF16, kind="ExternalInput").ap()
    out_d = nc.dram_tensor("out", [T, C], BF16, kind="ExternalOutput").ap()
    with tile.TileContext(nc) as tc:
        _emit(nc, tc, xT_d, wqk_d, wv_d, wp_d, out_d)
    nc.compile()
    return nc


def _host_inputs(x, Wqkv, Wproj):
    in_maps = []
    for c in range(8):
        b, g = divmod(c, 4)
        hs = [3 * g, 3 * g + 1, 3 * g + 2]

        def qcol(h):
            return Wqkv[:, 64 * h : 64 * h + 64]

        def kcol(h):
            return Wqkv[:, C + 64 * h : C + 64 * h + 64]

        def vcol(h):
            return Wqkv[:, 2 * C + 64 * h : 2 * C + 64 * h + 64]

        wqk = np.concatenate(
            [
                qcol(hs[0]), qcol(hs[1]),
                kcol(hs[0]), kcol(hs[1]),
                qcol(hs[2]), kcol(hs[2]),
            ],
            axis=1,
        )
        wv = np.concatenate([vcol(hs[0]), vcol(hs[1]), vcol(hs[2])], axis=1)
        wp = Wproj[VW * g : VW * (g + 1), :]
        in_maps.append(
            {
                "xt": np.ascontiguousarray(x[b].T).astype(BF16NP),
                "wqk": np.ascontiguousarray(wqk).astype(BF16NP),
                "wv": np.ascontiguousarray(wv).astype(BF16NP),
                "wp": np.ascontiguousarray(wp).astype(BF16NP),
            }
        )
    return in_maps


def kernel(x, mask, Wqkv, Wproj):
    global LAST_RESULT
    x = np.asarray(x, dtype=np.float32)
    Wqkv = np.asarray(Wqkv, dtype=np.float32)
    Wproj = np.asarray(Wproj, dtype=np.float32)

    in_maps = _host_inputs(x, Wqkv, Wproj)
    nc = _build()
    res = run_bass_kernel_spmd(nc, in_maps, core_ids=list(range(8)))
    LAST_RESULT = res
    out = np.empty((B, T, C), dtype=np.float32)
    for b in range(B):
        acc = res.results[4 * b]["out"].astype(np.float32)
        for g in range(1, 4):
            acc = acc + res.results[4 * b + g]["out"].astype(np.float32)
        out[b] = acc
    return out


if __name__ == "__main__":
    rng = np.random.default_rng(0)
    x = rng.standard_normal((B, T, C), dtype=np.float32)
    wqkv = rng.standard_normal((C, 3 * C), dtype=np.float32) / np.sqrt(C)
    wproj = rng.standard_normal((C, C), dtype=np.float32) / np.sqrt(C)
    o = kernel(x, None, wqkv, wproj)
    print(o.shape, o.dtype)


# revision 41
# speedup vs baseline: 1.0483x; 1.0483x over previous
"""Causal self-attention Trainium2 kernel (8 NeuronCores), v7.

Sharding: data-parallel over batch (2) x tensor-parallel over head groups
(12 heads -> 4 groups of 3). Core c handles batch c//4, head group c%4.
Each core computes its partial projection output (bf16); the host sums
the 4 partials per batch (TP reduce folded into the output gather).

All-bf16 compute.  Measured on HW: matmuls with full 128-row stationary
and 128 moving partitions stream at ~0.42ns/col with LDWEIGHTS fully
hidden inside the previous matmul; narrow shapes (K=64 QK, M=65 AV)
average 1.3-1.5x that.  v7 therefore:
  - pads the AV stationary to M=128 (v tiles stored 128 wide: 64 v cols,
    ones col at 64, zeros above -- extra PSUM rows are exact zeros).
  - pads the denominator-broadcast stationary to M=128 as well.
  - QK starts ~3us in: input DMAs issue the first 512 columns of x
    first, rows 0-1 are chunked at 512, and the prelude only needs two
    qkv groups; the ACT exp table is preloaded during the DMA wait.
  - softmax normalize: DVE den copy (row 64, lane-aligned) -> PE
    broadcast (f32r) into a y-pool PSUM tile -> DVE reciprocal -> DVE
    multiply.  DVE ops never move data across partitions (hardware
    cannot; CoreSim would not catch it).
  - AV(3) pre-accumulated over rows 0..13 in 3 of the 4 y-pool buffers;
    after the last exp only 2 small matmuls per head + proj 12-15
    remain, with half the tail casts on the then-idle ScalarE.
"""

import functools

import numpy as np
import ml_dtypes

import concourse.bass as bass
import concourse.mybir as mybir
import concourse.tile as tile
from concourse import bacc
from concourse.bass_utils import run_bass_kernel_spmd
from concourse.masks import make_upper_triangular

P = 128
B, T, C = 2, 2048, 768
NH, HD = 12, 64
HPG = 3              # heads per core
NT = T // P          # 16 key tiles
NQ = T // 512        # 4 query chunks
QKW = 2 * HPG * HD   # 384 qk channels per core
VW = HPG * HD        # 192 v channels per core
F32 = mybir.dt.float32
F32R = mybir.dt.float32r
BF16 = mybir.dt.bfloat16
BF16NP = ml_dtypes.bfloat16

W = [T - P * j for j in range(NT)]
R3OFF = []
_o = 0
for _j in range(NT):
    R3OFF.append(_o)
    _o += HPG * W[_j]
EXPTOT = _o  # 52224

LAST_RESULT = None


def _emit(nc, tc, xT_d, wqk_d, wv_d, wp_d, out_d):
    from contextlib import ExitStack

    ctx = ExitStack()
    with ctx:
        const = ctx.enter_context(tc.tile_pool(name="const", bufs=1))
        tri_b = const.tile([P, P], BF16)
        make_upper_triangular(nc, tri_b[:], val=1.0, diag=True)
        tri3 = const.tile([P, HPG * P], BF16)
        for h in range(HPG):
            nc.vector.tensor_copy(
                out=tri3[:, h * P : (h + 1) * P], in_=tri_b[:]
            )
        ones_f32 = const.tile([P, HD], F32)
        nc.any.memset(ones_f32[:], 1.0)
        ones64 = const.tile([P, HD], F32R)
        nc.vector.tensor_copy(out=ones64[:], in_=ones_f32[:])

        # ---- weights + xT in SBUF -------------------------------------
        w_pool = ctx.enter_context(tc.tile_pool(name="w", bufs=1))
        wqk_sb = []
        for cc in range(6):
            t = w_pool.tile([P, QKW], BF16, tag=f"wqk{cc}")
            nc.sync.dma_start(t[:], wqk_d[cc * P : (cc + 1) * P, :])
            wqk_sb.append(t)
        x_pool = ctx.enter_context(tc.tile_pool(name="x", bufs=1))
        xT_sb = []
        for cc in range(6):
            t = x_pool.tile([P, T], BF16, tag=f"x{cc}", name=f"x{cc}")
            xT_sb.append(t)
        for cc in range(6):
            nc.sync.dma_start(
                xT_sb[cc][:, 0:1024], xT_d[cc * P : (cc + 1) * P, 0:1024]
            )
        wv_sb = []
        for cc in range(6):
            t = w_pool.tile([P, VW], BF16, tag=f"wv{cc}")
            nc.sync.dma_start(t[:], wv_d[cc * P : (cc + 1) * P, :])
            wv_sb.append(t)
        for cc in range(6):
            nc.sync.dma_start(
                xT_sb[cc][:, 1024:T], xT_d[cc * P : (cc + 1) * P, 1024:T]
            )
        wpA = w_pool.tile([P, C], BF16, tag="wpA")
        nc.sync.dma_start(wpA[:], wp_d[0:P, :])
        wpB = w_pool.tile([HD, C], BF16, tag="wpB")
        nc.sync.dma_start(wpB[:], wp_d[P : P + HD, :])

        # ---- persistent SBUF tensors ----------------------------------
        big = ctx.enter_context(tc.tile_pool(name="big", bufs=1))
        qkT = [
            big.tile([P, T], BF16, tag=f"qkT{m}", name=f"qkT{m}")
            for m in range(3)
        ]
        k2b = big.tile([HD, T], BF16, tag="k2b")
        v_sb = big.tile([P, HPG * NT * (HD + 1)], BF16, tag="vsb")
        nc.any.memset(v_sb[:], 1.0)  # ones cols at 64 mod 65 survive
        vv = v_sb[:].rearrange(
            "p (h t d) -> p h t d", h=HPG, t=NT, d=HD + 1
        )
        exp_sb = big.tile([P, EXPTOT], BF16, tag="exp")
        yT_a = big.tile([P, T], BF16, tag="ya")   # h0 rows 0:64, h1 64:128
        yT_b = big.tile([HD, T], BF16, tag="yb")  # h2

        nrm_pool = ctx.enter_context(tc.tile_pool(name="nrm", bufs=2))
        out_pool = ctx.enter_context(tc.tile_pool(name="outp", bufs=3))

        def blk(j):
            return exp_sb[:, R3OFF[j] : R3OFF[j] + HPG * W[j]].rearrange(
                "p (h n) -> p h n", h=HPG
            )

        q_sl = [(qkT[0], 0), (qkT[0], HD), (qkT[2], 0)]
        k_sl = [(qkT[1], 0), (qkT[1], HD), (k2b, 0)]

        def ydst_of(h):
            return yT_a[0:HD, :] if h == 0 else (
                yT_a[HD:P, :] if h == 1 else yT_b[0:HD, :]
            )

        # ---------------- emission helpers ----------------
        ps_att = ctx.enter_context(
            tc.tile_pool(name="ps_att", bufs=1, space="PSUM")
        )

        def emit_qkv_group(ps_b, m, g):
            ps = ps_b.tile([P, 512], F32, tag="ab", bufs=2)
            for cc in range(6):
                nc.tensor.matmul(
                    ps[:],
                    wqk_sb[cc][:, m * P : (m + 1) * P],
                    xT_sb[cc][:, g * 512 : (g + 1) * 512],
                    start=(cc == 0),
                    stop=(cc == 5),
                )
            nc.vector.tensor_copy(
                out=qkT[m][:, g * 512 : (g + 1) * 512], in_=ps[:]
            )
            if m == 2:
                nc.sync.dma_start(
                    k2b[:, g * 512 : (g + 1) * 512],
                    qkT[2][HD:P, g * 512 : (g + 1) * 512],
                )

        def emit_v_group(ps_b, tt):
            ps = ps_b.tile([P, 512], F32, tag="ab", bufs=2)
            for cc in range(6):
                nc.tensor.matmul(
                    ps[:, 0:VW],
                    xT_sb[cc][:, tt * P : (tt + 1) * P],
                    wv_sb[cc][:, :],
                    start=(cc == 0),
                    stop=(cc == 5),
                )
            dst = vv[:, :, tt, 0:HD]
            src = ps[:, 0:VW].rearrange("p (h d) -> p h d", h=HPG)
            nc.vector.tensor_copy(out=dst, in_=src)

        def emit_qk_unit(j, h, off, cw):
            qlo = P * j + off
            qt, qo = q_sl[h]
            kt, ko = k_sl[h]
            st = ps_att.tile([P, 1024], F32, tag="st", bufs=2)
            for s0 in range(0, cw, 512):
                sw = min(512, cw - s0)
                nc.tensor.matmul(
                    st[:, s0 : s0 + sw],
                    kt[ko : ko + HD, P * j : P * j + P],
                    qt[qo : qo + HD, qlo + s0 : qlo + s0 + sw],
                    start=True,
                    stop=True,
                )
            dst = blk(j)[:, h, off : off + cw]
            nc.scalar.activation(
                dst, st[:, 0:cw], mybir.ActivationFunctionType.Exp,
                scale=0.125,
            )

        def emit_mask(j):
            dg = blk(j)[:, :, 0:P]
            t3 = tri3[:].rearrange("p (h n) -> p h n", h=HPG)
            nc.gpsimd.tensor_mul(out=dg, in0=dg, in1=t3)

        def vtile(h, jj):
            a = (h * NT + jj) * (HD + 1)
            return v_sb[:, a : a + (HD + 1)]

        def emit_av_mm(yq, q, h, jj, is_first, is_last):
            woff = 512 * q - P * jj
            lo = max(0, -woff)
            n = 512 - lo
            rhs = blk(jj)[:, h, woff + lo : woff + lo + n]
            nc.tensor.matmul(
                yq[0 : HD + 1, lo : lo + n],
                vtile(h, jj),
                rhs,
                start=is_first,
                stop=is_last,
            )

        def emit_norm(yq, q, h):
            den = nrm_pool.tile([P, 512], F32R, tag="den")
            nc.vector.tensor_copy(
                out=den[HD : HD + 1, :], in_=yq[HD : HD + 1, 0:512]
            )
            bc = ps_c.tile([P, 512], F32, tag="y", bufs=4)
            nc.tensor.matmul(
                bc[0:HD, :],
                ones64[HD : HD + 1, :],
                den[HD : HD + 1, :],
                start=True,
                stop=True,
            )
            bcs = nrm_pool.tile([HD, 512], F32, tag="bcs")
            with nc.allow_low_precision(reason="softmax denom"):
                nc.vector.reciprocal_approx_fast(bcs[:], bc[0:HD, :])
            nc.vector.tensor_mul(
                out=ydst_of(h)[:, 512 * q : 512 * (q + 1)],
                in0=yq[0:HD, 0:512],
                in1=bcs[:],
            )

        def emit_av(ps_c, q, h):
            yq = ps_c.tile([P, 512], F32, tag="y", bufs=4)
            for jj in range(4 * q + 4):
                emit_av_mm(yq, q, h, jj, jj == 0, jj == 4 * q + 3)
            emit_norm(yq, q, h)

        def emit_proj(ps_c, tt, cast_engine="dve"):
            pja = ps_c.tile([P, 512], F32, tag="y", bufs=4)
            pjb = ps_c.tile([P, 512], F32, tag="y", bufs=4)
            ysl = slice(tt * P, (tt + 1) * P)
            nc.tensor.matmul(
                pja[:], yT_a[:, ysl], wpA[:, 0:512], start=True, stop=False
            )
            nc.tensor.matmul(
                pjb[:, 0:256], yT_a[:, ysl], wpA[:, 512:C],
                start=True, stop=False,
            )
            nc.tensor.matmul(
                pja[:], yT_b[:, ysl], wpB[:, 0:512], start=False, stop=True
            )
            nc.tensor.matmul(
                pjb[:, 0:256], yT_b[:, ysl], wpB[:, 512:C],
                start=False, stop=True,
            )
            ot = out_pool.tile([P, C], BF16, tag="o")
            if cast_engine == "act":
                nc.scalar.copy(out=ot[:, 0:512], in_=pja[:])
                nc.scalar.copy(out=ot[:, 512:C], in_=pjb[:, 0:256])
            else:
                nc.vector.tensor_copy(out=ot[:, 0:512], in_=pja[:])
                nc.vector.tensor_copy(out=ot[:, 512:C], in_=pjb[:, 0:256])
            nc.sync.dma_start(out_d[tt * P : (tt + 1) * P, :], ot[:])

        # ---------------- schedule ----------------
        ps_b = tc.alloc_tile_pool(name="ps_b", bufs=1, space="PSUM")
        ps_c = None

        emit_qkv_group(ps_b, 0, 0)
        emit_qkv_group(ps_b, 1, 0)
        emit_qkv_group(ps_b, 2, 0)

        fillers = []
        for g in (1, 2, 3):
            fillers.append(
                (1340, f"m0g{g}", lambda g=g: emit_qkv_group(ps_b, 0, g))
            )
            fillers.append(
                (1340, f"m2g{g}", lambda g=g: emit_qkv_group(ps_b, 2, g))
            )
            fillers.append(
                (1340, f"m1g{g}", lambda g=g: emit_qkv_group(ps_b, 1, g))
            )
        for tt in range(NT):
            fillers.append(
                (480, f"v{tt}", lambda tt=tt: emit_v_group(ps_b, tt))
            )
        fi = 0
        done_tags = set()

        def pop_filler():
            nonlocal fi
            cost, tag, fn = fillers[fi]
            fi += 1
            fn()
            done_tags.add(tag)
            return cost

        def need(tag):
            while tag not in done_tags and fi < len(fillers):
                pop_filler()

        def drain():
            while fi < len(fillers):
                pop_filler()

        for j in range(NT):
            csz = 1024
            offs = []
            o = 0
            while o < W[j]:
                offs.append((o, min(csz, W[j] - o)))
                o += csz
            for off, cw in offs:
                g_need = (P * j + off + cw - 1) // 512
                for g in range(1, g_need + 1):
                    need(f"m0g{g}")
                    need(f"m2g{g}")
                kg = j // 4
                if kg >= 1:
                    need(f"m1g{kg}")
                for h in range(HPG):
                    emit_qk_unit(j, h, off, cw)
                    if off == 0 and h == 2:
                        emit_mask(j)
                    slack = cw * 0.417 + 220
                    while slack > 0 and fi < len(fillers):
                        slack -= pop_filler()
            if j == 3:
                drain()
                ps_b.release()
                ps_c = tc.alloc_tile_pool(name="ps_c", bufs=1, space="PSUM")
            if j in (3, 7, 11):
                q = j // 4
                cost = (4 * q + 2) * 215
                for h in range(HPG):
                    fillers.append(
                        (cost, f"av{q}h{h}",
                         lambda q=q, h=h: emit_av(ps_c, q, h))
                    )
            if j in (3, 9, 13):
                q = max(0, (j - 5) // 4)
                for t4 in range(4):
                    fillers.append(
                        (660, f"pj{4 * q + t4}",
                         lambda q=q, t4=t4: emit_proj(ps_c, 4 * q + t4))
                    )
            if j == 13:
                drain()
                # pre-accumulate AV(3) over rows 0..13 in 3 y-pool bufs
                yq3 = []
                for h in range(HPG):
                    yq = ps_c.tile([P, 512], F32, tag="y", bufs=4)
                    for jj in range(14):
                        emit_av_mm(yq, 3, h, jj, jj == 0, False)
                    yq3.append(yq)

        # ---- tail: finish AV(3), last projection tiles ----
        for h in range(HPG):
            emit_av_mm(yq3[h], 3, h, 14, False, False)
            emit_av_mm(yq3[h], 3, h, 15, False, True)
            emit_norm(yq3[h], 3, h)
        emit_proj(ps_c, 12, cast_engine="dve")
        emit_proj(ps_c, 13, cast_engine="act")
        emit_proj(ps_c, 14, cast_engine="dve")
        emit_proj(ps_c, 15, cast_engine="act")
        ps_c.release()


@functools.cache
def _build():
    nc = bacc.Bacc(
        "TRN2",
        target_bir_lowering=False,
        debug=False,
        enable_asserts=False,
        num_devices=8,
    )
    xT_d = nc.dram_tensor("xt", [C, T], BF16, kind="ExternalInput").ap()
    wqk_d = nc.dram_tensor("wqk", [C, QKW], BF16, kind="ExternalInput").ap()
    wv_d = nc.dram_tensor("wv", [C, VW], BF16, kind="ExternalInput").ap()
    wp_d = nc.dram_tensor("wp", [VW, C], BF16, kind="ExternalInput").ap()
    out_d = nc.dram_tensor("out", [T, C], BF16, kind="ExternalOutput").ap()
    with tile.TileContext(nc) as tc:
        _emit(nc, tc, xT_d, wqk_d, wv_d, wp_d, out_d)
    nc.compile()
    return nc


def _host_inputs(x, Wqkv, Wproj):
    in_maps = []
    for c in range(8):
        b, g = divmod(c, 4)
        hs = [3 * g, 3 * g + 1, 3 * g + 2]

        def qcol(h):
            return Wqkv[:, 64 * h : 64 * h + 64]

        def kcol(h):
            return Wqkv[:, C + 64 * h : C + 64 * h + 64]

        def vcol(h):
            return Wqkv[:, 2 * C + 64 * h : 2 * C + 64 * h + 64]

        wqk = np.concatenate(
            [
                qcol(hs[0]), qcol(hs[1]),
                kcol(hs[0]), kcol(hs[1]),
                qcol(hs[2]), kcol(hs[2]),
            ],
            axis=1,
        )
        wv = np.concatenate([vcol(hs[0]), vcol(hs[1]), vcol(hs[2])], axis=1)
        wp = Wproj[VW * g : VW * (g + 1), :]
        in_maps.append(
            {
                "xt": np.ascontiguousarray(x[b].T).astype(BF16NP),
                "wqk": np.ascontiguousarray(wqk).astype(BF16NP),
                "wv": np.ascontiguousarray(wv).astype(BF16NP),
                "wp": np.ascontiguousarray(wp).astype(BF16NP),
            }
        )
    return in_maps


def kernel(x, mask, Wqkv, Wproj):
    global LAST_RESULT
    x = np.asarray(x, dtype=np.float32)
    Wqkv = np.asarray(Wqkv, dtype=np.float32)
    Wproj = np.asarray(Wproj, dtype=np.float32)

    in_maps = _host_inputs(x, Wqkv, Wproj)
    nc = _build()
    res = run_bass_kernel_spmd(nc, in_maps, core_ids=list(range(8)))
    LAST_RESULT = res
    out = np.empty((B, T, C), dtype=np.float32)
    for b in range(B):
        acc = res.results[4 * b]["out"].astype(np.float32)
        for g in range(1, 4):
            acc = acc + res.results[4 * b + g]["out"].astype(np.float32)
        out[b] = acc
    return out


if __name__ == "__main__":
    rng = np.random.default_rng(0)
    x = rng.standard_normal((B, T, C), dtype=np.float32)
    wqkv = rng.standard_normal((C, 3 * C), dtype=np.float32) / np.sqrt(C)
    wproj = rng.standard_normal((C, C), dtype=np.float32) / np.sqrt(C)
    o = kernel(x, None, wqkv, wproj)
    print(o.shape, o.dtype)


# revision 42
# speedup vs baseline: 1.0569x; 1.0081x over previous
"""Causal self-attention Trainium2 kernel (8 NeuronCores), v7.

Sharding: data-parallel over batch (2) x tensor-parallel over head groups
(12 heads -> 4 groups of 3). Core c handles batch c//4, head group c%4.
Each core computes its partial projection output (bf16); the host sums
the 4 partials per batch (TP reduce folded into the output gather).

All-bf16 compute.  Measured on HW: matmuls with full 128-row stationary
and 128 moving partitions stream at ~0.42ns/col with LDWEIGHTS fully
hidden inside the previous matmul; narrow shapes (K=64 QK, M=65 AV)
average 1.3-1.5x that.  v7 therefore:
  - pads the AV stationary to M=128 (v tiles stored 128 wide: 64 v cols,
    ones col at 64, zeros above -- extra PSUM rows are exact zeros).
  - pads the denominator-broadcast stationary to M=128 as well.
  - QK starts ~3us in: input DMAs issue the first 512 columns of x
    first, rows 0-1 are chunked at 512, and the prelude only needs two
    qkv groups; the ACT exp table is preloaded during the DMA wait.
  - softmax normalize: DVE den copy (row 64, lane-aligned) -> PE
    broadcast (f32r) into a y-pool PSUM tile -> DVE reciprocal -> DVE
    multiply.  DVE ops never move data across partitions (hardware
    cannot; CoreSim would not catch it).
  - AV(3) pre-accumulated over rows 0..13 in 3 of the 4 y-pool buffers;
    after the last exp only 2 small matmuls per head + proj 12-15
    remain, with half the tail casts on the then-idle ScalarE.
"""

import functools

import numpy as np
import ml_dtypes

import concourse.bass as bass
import concourse.mybir as mybir
import concourse.tile as tile
from concourse import bacc
from concourse.bass_utils import run_bass_kernel_spmd
from concourse.masks import make_upper_triangular

P = 128
B, T, C = 2, 2048, 768
NH, HD = 12, 64
HPG = 3              # heads per core
NT = T // P          # 16 key tiles
NQ = T // 512        # 4 query chunks
QKW = 2 * HPG * HD   # 384 qk channels per core
VW = HPG * HD        # 192 v channels per core
F32 = mybir.dt.float32
F32R = mybir.dt.float32r
BF16 = mybir.dt.bfloat16
BF16NP = ml_dtypes.bfloat16

W = [T - P * j for j in range(NT)]
R3OFF = []
_o = 0
for _j in range(NT):
    R3OFF.append(_o)
    _o += HPG * W[_j]
EXPTOT = _o  # 52224

LAST_RESULT = None


def _emit(nc, tc, xT_d, wqk_d, wv_d, wp_d, out_d):
    from contextlib import ExitStack

    ctx = ExitStack()
    with ctx:
        const = ctx.enter_context(tc.tile_pool(name="const", bufs=1))
        tri_b = const.tile([P, P], BF16)
        make_upper_triangular(nc, tri_b[:], val=1.0, diag=True)
        tri3 = const.tile([P, HPG * P], BF16)
        for h in range(HPG):
            nc.vector.tensor_copy(
                out=tri3[:, h * P : (h + 1) * P], in_=tri_b[:]
            )
        ones_f32 = const.tile([P, HD], F32)
        nc.any.memset(ones_f32[:], 1.0)
        ones64 = const.tile([P, HD], F32R)
        nc.vector.tensor_copy(out=ones64[:], in_=ones_f32[:])

        # ---- weights + xT in SBUF -------------------------------------
        w_pool = ctx.enter_context(tc.tile_pool(name="w", bufs=1))
        wqk_sb = []
        for cc in range(6):
            t = w_pool.tile([P, QKW], BF16, tag=f"wqk{cc}")
            nc.sync.dma_start(t[:], wqk_d[cc * P : (cc + 1) * P, :])
            wqk_sb.append(t)
        x_pool = ctx.enter_context(tc.tile_pool(name="x", bufs=1))
        xT_sb = []
        for cc in range(6):
            t = x_pool.tile([P, T], BF16, tag=f"x{cc}", name=f"x{cc}")
            xT_sb.append(t)
        for cc in range(6):
            nc.sync.dma_start(
                xT_sb[cc][:, 0:1024], xT_d[cc * P : (cc + 1) * P, 0:1024]
            )
        wv_sb = []
        for cc in range(6):
            t = w_pool.tile([P, VW], BF16, tag=f"wv{cc}")
            nc.sync.dma_start(t[:], wv_d[cc * P : (cc + 1) * P, :])
            wv_sb.append(t)
        for cc in range(6):
            nc.sync.dma_start(
                xT_sb[cc][:, 1024:T], xT_d[cc * P : (cc + 1) * P, 1024:T]
            )
        wpA = w_pool.tile([P, C], BF16, tag="wpA")
        nc.sync.dma_start(wpA[:], wp_d[0:P, :])
        wpB = w_pool.tile([HD, C], BF16, tag="wpB")
        nc.sync.dma_start(wpB[:], wp_d[P : P + HD, :])

        # ---- persistent SBUF tensors ----------------------------------
        big = ctx.enter_context(tc.tile_pool(name="big", bufs=1))
        qkT = [
            big.tile([P, T], BF16, tag=f"qkT{m}", name=f"qkT{m}")
            for m in range(3)
        ]
        k2b = big.tile([HD, T], BF16, tag="k2b")
        v_sb = big.tile([P, HPG * NT * (HD + 1)], BF16, tag="vsb")
        nc.any.memset(v_sb[:], 1.0)  # ones cols at 64 mod 65 survive
        vv = v_sb[:].rearrange(
            "p (h t d) -> p h t d", h=HPG, t=NT, d=HD + 1
        )
        exp_sb = big.tile([P, EXPTOT], BF16, tag="exp")
        yT_a = big.tile([P, T], BF16, tag="ya")   # h0 rows 0:64, h1 64:128
        yT_b = big.tile([HD, T], BF16, tag="yb")  # h2

        nrm_pool = ctx.enter_context(tc.tile_pool(name="nrm", bufs=2))
        out_pool = ctx.enter_context(tc.tile_pool(name="outp", bufs=3))

        def blk(j):
            return exp_sb[:, R3OFF[j] : R3OFF[j] + HPG * W[j]].rearrange(
                "p (h n) -> p h n", h=HPG
            )

        q_sl = [(qkT[0], 0), (qkT[0], HD), (qkT[2], 0)]
        k_sl = [(qkT[1], 0), (qkT[1], HD), (k2b, 0)]

        def ydst_of(h):
            return yT_a[0:HD, :] if h == 0 else (
                yT_a[HD:P, :] if h == 1 else yT_b[0:HD, :]
            )

        # ---------------- emission helpers ----------------
        ps_att = ctx.enter_context(
            tc.tile_pool(name="ps_att", bufs=1, space="PSUM")
        )

        def emit_qkv_group(ps_b, m, g):
            ps = ps_b.tile([P, 512], F32, tag="ab", bufs=2)
            for cc in range(6):
                nc.tensor.matmul(
                    ps[:],
                    wqk_sb[cc][:, m * P : (m + 1) * P],
                    xT_sb[cc][:, g * 512 : (g + 1) * 512],
                    start=(cc == 0),
                    stop=(cc == 5),
                )
            nc.vector.tensor_copy(
                out=qkT[m][:, g * 512 : (g + 1) * 512], in_=ps[:]
            )
            if m == 2:
                nc.sync.dma_start(
                    k2b[:, g * 512 : (g + 1) * 512],
                    qkT[2][HD:P, g * 512 : (g + 1) * 512],
                )

        def emit_v_group(ps_b, tt):
            ps = ps_b.tile([P, 512], F32, tag="ab", bufs=2)
            for cc in range(6):
                nc.tensor.matmul(
                    ps[:, 0:VW],
                    xT_sb[cc][:, tt * P : (tt + 1) * P],
                    wv_sb[cc][:, :],
                    start=(cc == 0),
                    stop=(cc == 5),
                )
            dst = vv[:, :, tt, 0:HD]
            src = ps[:, 0:VW].rearrange("p (h d) -> p h d", h=HPG)
            nc.vector.tensor_copy(out=dst, in_=src)

        def emit_qk_unit(j, h, off, cw):
            qlo = P * j + off
            qt, qo = q_sl[h]
            kt, ko = k_sl[h]
            st = ps_att.tile([P, 1024], F32, tag="st", bufs=2)
            for s0 in range(0, cw, 512):
                sw = min(512, cw - s0)
                nc.tensor.matmul(
                    st[:, s0 : s0 + sw],
                    kt[ko : ko + HD, P * j : P * j + P],
                    qt[qo : qo + HD, qlo + s0 : qlo + s0 + sw],
                    start=True,
                    stop=True,
                )
            dst = blk(j)[:, h, off : off + cw]
            nc.scalar.activation(
                dst, st[:, 0:cw], mybir.ActivationFunctionType.Exp,
                scale=0.125,
            )

        def emit_mask(j):
            dg = blk(j)[:, :, 0:P]
            t3 = tri3[:].rearrange("p (h n) -> p h n", h=HPG)
            nc.gpsimd.tensor_mul(out=dg, in0=dg, in1=t3)

        def vtile(h, jj):
            a = (h * NT + jj) * (HD + 1)
            return v_sb[:, a : a + (HD + 1)]

        def emit_av_mm(yq, q, h, jj, is_first, is_last):
            woff = 512 * q - P * jj
            lo = max(0, -woff)
            n = 512 - lo
            rhs = blk(jj)[:, h, woff + lo : woff + lo + n]
            nc.tensor.matmul(
                yq[0 : HD + 1, lo : lo + n],
                vtile(h, jj),
                rhs,
                start=is_first,
                stop=is_last,
            )

        def emit_norm(yq, q, h):
            den = nrm_pool.tile([P, 512], F32R, tag="den")
            nc.vector.tensor_copy(
                out=den[HD : HD + 1, :], in_=yq[HD : HD + 1, 0:512]
            )
            bc = ps_c.tile([P, 512], F32, tag="y", bufs=4)
            nc.tensor.matmul(
                bc[0:HD, :],
                ones64[HD : HD + 1, :],
                den[HD : HD + 1, :],
                start=True,
                stop=True,
            )
            bcs = nrm_pool.tile([HD, 512], F32, tag="bcs")
            with nc.allow_low_precision(reason="softmax denom"):
                nc.vector.reciprocal_approx_fast(bcs[:], bc[0:HD, :])
            nc.vector.tensor_mul(
                out=ydst_of(h)[:, 512 * q : 512 * (q + 1)],
                in0=yq[0:HD, 0:512],
                in1=bcs[:],
            )

        def emit_av(ps_c, q, h):
            yq = ps_c.tile([P, 512], F32, tag="y", bufs=4)
            for jj in range(4 * q + 4):
                emit_av_mm(yq, q, h, jj, jj == 0, jj == 4 * q + 3)
            emit_norm(yq, q, h)

        def emit_proj(ps_c, tt, cast_engine="dve"):
            pja = ps_c.tile([P, 512], F32, tag="y", bufs=4)
            pjb = ps_c.tile([P, 512], F32, tag="y", bufs=4)
            ysl = slice(tt * P, (tt + 1) * P)
            nc.tensor.matmul(
                pja[:], yT_a[:, ysl], wpA[:, 0:512], start=True, stop=False
            )
            nc.tensor.matmul(
                pjb[:, 0:256], yT_a[:, ysl], wpA[:, 512:C],
                start=True, stop=False,
            )
            nc.tensor.matmul(
                pja[:], yT_b[:, ysl], wpB[:, 0:512], start=False, stop=True
            )
            nc.tensor.matmul(
                pjb[:, 0:256], yT_b[:, ysl], wpB[:, 512:C],
                start=False, stop=True,
            )
            ot = out_pool.tile([P, C], BF16, tag="o")
            if cast_engine == "act":
                nc.scalar.copy(out=ot[:, 0:512], in_=pja[:])
                nc.scalar.copy(out=ot[:, 512:C], in_=pjb[:, 0:256])
            else:
                nc.vector.tensor_copy(out=ot[:, 0:512], in_=pja[:])
                nc.vector.tensor_copy(out=ot[:, 512:C], in_=pjb[:, 0:256])
            nc.sync.dma_start(out_d[tt * P : (tt + 1) * P, :], ot[:])

        # ---------------- schedule ----------------
        ps_b = tc.alloc_tile_pool(name="ps_b", bufs=1, space="PSUM")
        ps_c = None

        emit_qkv_group(ps_b, 0, 0)
        emit_qkv_group(ps_b, 1, 0)
        emit_qkv_group(ps_b, 2, 0)

        fillers = []
        for g in (1, 2, 3):
            fillers.append(
                (1340, f"m0g{g}", lambda g=g: emit_qkv_group(ps_b, 0, g))
            )
            fillers.append(
                (1340, f"m2g{g}", lambda g=g: emit_qkv_group(ps_b, 2, g))
            )
            fillers.append(
                (1340, f"m1g{g}", lambda g=g: emit_qkv_group(ps_b, 1, g))
            )
        for tt in range(NT):
            fillers.append(
                (480, f"v{tt}", lambda tt=tt: emit_v_group(ps_b, tt))
            )
        fi = 0
        done_tags = set()

        def pop_filler():
            nonlocal fi
            cost, tag, fn = fillers[fi]
            fi += 1
            fn()
            done_tags.add(tag)
            return cost

        def need(tag):
            while tag not in done_tags and fi < len(fillers):
                pop_filler()

        def drain():
            while fi < len(fillers):
                pop_filler()

        for j in range(NT):
            csz = 1024
            offs = []
            o = 0
            while o < W[j]:
                offs.append((o, min(csz, W[j] - o)))
                o += csz
            for off, cw in offs:
                g_need = (P * j + off + cw - 1) // 512
                for g in range(1, g_need + 1):
                    need(f"m0g{g}")
                    need(f"m2g{g}")
                kg = j // 4
                if kg >= 1:
                    need(f"m1g{kg}")
                for h in range(HPG):
                    emit_qk_unit(j, h, off, cw)
                    if off == 0 and h == 2:
                        emit_mask(j)
                    slack = cw * 0.417 + 220
                    while slack > 0 and fi < len(fillers):
                        slack -= pop_filler()
            if j == 3:
                drain()
                ps_b.release()
                ps_c = tc.alloc_tile_pool(name="ps_c", bufs=1, space="PSUM")
            if j in (3, 7, 11):
                q = j // 4
                cost = (4 * q + 2) * 215
                for h in range(HPG):
                    fillers.append(
                        (cost, f"av{q}h{h}",
                         lambda q=q, h=h: emit_av(ps_c, q, h))
                    )
                for t4 in range(4):
                    fillers.append(
                        (660, f"pj{4 * q + t4}",
                         lambda q=q, t4=t4: emit_proj(ps_c, 4 * q + t4))
                    )
            if j == 13:
                drain()
                # pre-accumulate AV(3) over rows 0..13 in 3 y-pool bufs
                yq3 = []
                for h in range(HPG):
                    yq = ps_c.tile([P, 512], F32, tag="y", bufs=4)
                    for jj in range(14):
                        emit_av_mm(yq, 3, h, jj, jj == 0, False)
                    yq3.append(yq)

        # ---- tail: finish AV(3), last projection tiles ----
        for h in range(HPG):
            emit_av_mm(yq3[h], 3, h, 14, False, False)
            emit_av_mm(yq3[h], 3, h, 15, False, True)
            emit_norm(yq3[h], 3, h)
        emit_proj(ps_c, 12, cast_engine="dve")
        emit_proj(ps_c, 13, cast_engine="act")
        emit_proj(ps_c, 14, cast_engine="dve")
        emit_proj(ps_c, 15, cast_engine="act")
        ps_c.release()


@functools.cache
def _build():
    nc = bacc.Bacc(
        "TRN2",
        target_bir_lowering=False,
        debug=False,
        enable_asserts=False,
        num_devices=8,
    )
    xT_d = nc.dram_tensor("xt", [C, T], BF16, kind="ExternalInput").ap()
    wqk_d = nc.dram_tensor("wqk", [C, QKW], BF16, kind="ExternalInput").ap()
    wv_d = nc.dram_tensor("wv", [C, VW], BF16, kind="ExternalInput").ap()
    wp_d = nc.dram_tensor("wp", [VW, C], BF16, kind="ExternalInput").ap()
    out_d = nc.dram_tensor("out", [T, C], BF16, kind="ExternalOutput").ap()
    with tile.TileContext(nc) as tc:
        _emit(nc, tc, xT_d, wqk_d, wv_d, wp_d, out_d)
    nc.compile()
    return nc


def _host_inputs(x, Wqkv, Wproj):
    in_maps = []
    for c in range(8):
        b, g = divmod(c, 4)
        hs = [3 * g, 3 * g + 1, 3 * g + 2]

        def qcol(h):
            return Wqkv[:, 64 * h : 64 * h + 64]

        def kcol(h):
            return Wqkv[:, C + 64 * h : C + 64 * h + 64]

        def vcol(h):
            return Wqkv[:, 2 * C + 64 * h : 2 * C + 64 * h + 64]

        wqk = np.concatenate(
            [
                qcol(hs[0]), qcol(hs[1]),
                kcol(hs[0]), kcol(hs[1]),
                qcol(hs[2]), kcol(hs[2]),
            ],
            axis=1,
        )
        wv = np.concatenate([vcol(hs[0]), vcol(hs[1]), vcol(hs[2])], axis=1)
        wp = Wproj[VW * g : VW * (g + 1), :]
        in_maps.append(
            {
                "xt": np.ascontiguousarray(x[b].T).astype(BF16NP),
                "wqk": np.ascontiguousarray(wqk).astype(BF16NP),
                "wv": np.ascontiguousarray(wv).astype(BF16NP),
                "wp": np.ascontiguousarray(wp).astype(BF16NP),
            }
        )
    return in_maps


def kernel(x, mask, Wqkv, Wproj):
    global LAST_RESULT
    x = np.asarray(x, dtype=np.float32)
    Wqkv = np.asarray(Wqkv, dtype=np.float32)
    Wproj = np.asarray(Wproj, dtype=np.float32)

    in_maps = _host_inputs(x, Wqkv, Wproj)
    nc = _build()
    res = run_bass_kernel_spmd(nc, in_maps, core_ids=list(range(8)))
    LAST_RESULT = res
    out = np.empty((B, T, C), dtype=np.float32)
    for b in range(B):
        acc = res.results[4 * b]["out"].astype(np.float32)
        for g in range(1, 4):
            acc = acc + res.results[4 * b + g]["out"].astype(np.float32)
        out[b] = acc
    return out


if __name__ == "__main__":
    rng = np.random.default_rng(0)
    x = rng.standard_normal((B, T, C), dtype=np.float32)
    wqkv = rng.standard_normal((C, 3 * C), dtype=np.float32) / np.sqrt(C)
    wproj = rng.standard_normal((C, C), dtype=np.float32) / np.sqrt(C)
    o = kernel(x, None, wqkv, wproj)
    print(o.shape, o.dtype)
